# revision 13
# baseline (speedup 1.0000x reference)
"""Sliding-window causal self-attention (B=2, T=2048, C=1024, H=16, Dh=64,
window=256) + QKV/out projections, sharded over 8 NeuronCores as
data-parallel over B (2) x tensor-parallel over head groups (4 heads/core).

Layout strategy ("sT scheme"): scores are computed TRANSPOSED
(sT[k, q] = khT^T @ qhT) so the exp() activation writes P^T straight to
SBUF. The band mask is a post-exp 0/1 multiply on bf16 SBUF data. Row
sums come free from a ones-column appended to each head's V tile
(oT_ext[:, 64] = rowsum, q on partitions), so normalization is a
per-partition scalar multiply. The attention output is transposed back
with PE identity matmuls right before the out-proj.

Scheduling: input chunks are issued pairwise on the SP and ACT DMA
queues so the first QKV matmul starts ~3us in. The emission is
software-pipelined per query tile so the PE always has independent work
while the exp chain drains:
  [sT h0,h1] [transpose qt-1] [V qt+1] [sT h2,h3] [oT h0..3] [outproj qt-1]
QKV token-splits are emitted several iterations before their attention
consumers; RoPE runs on DVE (q side) and the otherwise-idle GpSimd
engine (k side) so the head-major repack is ready with slack.
"""

import math

import numpy as np

B = 2
T = 2048
C = 1024
H = 16
DH = 64
WINDOW = 256
HEADS_PER_CORE = 4
N_CORES = 8
QT = T // 128  # 16 query tiles of 128
FQ = HEADS_PER_CORE * DH  # 256 local features
VW = DH + 1  # per-head v columns incl the fused ones column
VROW = HEADS_PER_CORE * VW  # 260 v columns per key tile

_PROGRAM = None  # compile once per process


def _emit(nc, tc, aps, ctx):
    from concourse import mybir

    f32 = mybir.dt.float32
    bf16 = mybir.dt.bfloat16
    Exp = mybir.ActivationFunctionType.Exp

    xT, wT, woT, cos4, sin4, amask01, ident, y = (
        aps["xT"], aps["wT"], aps["woT"], aps["cos4"], aps["sin4"],
        aps["amask01"], aps["ident"], aps["y"],
    )

    consts = ctx.enter_context(tc.tile_pool(name="consts", bufs=1))
    stage = ctx.enter_context(tc.tile_pool(name="stage", bufs=1))
    pre = ctx.enter_context(tc.tile_pool(name="pre", bufs=8))
    tmp = ctx.enter_context(tc.tile_pool(name="tmp", bufs=2))
    work = ctx.enter_context(tc.tile_pool(name="work", bufs=6))
    osbp = ctx.enter_context(tc.tile_pool(name="osbp", bufs=2))
    asbp = ctx.enter_context(tc.tile_pool(name="asbp", bufs=2))
    ysbp = ctx.enter_context(tc.tile_pool(name="ysbp", bufs=3))
    small = ctx.enter_context(tc.tile_pool(name="small", bufs=4))
    pmm = ctx.enter_context(tc.tile_pool(name="pmm", bufs=2, space="PSUM"))
    pout = ctx.enter_context(tc.tile_pool(name="pout", bufs=2, space="PSUM"))
    ps = ctx.enter_context(tc.tile_pool(name="ps", bufs=2, space="PSUM"))
    po = ctx.enter_context(tc.tile_pool(name="po", bufs=2, space="PSUM"))

    # ---- resident inputs ----
    xT_sb = consts.tile([128, 8 * T], bf16, tag="xT")  # [C-part, (kc t)]
    wT_sb = consts.tile([128, 8 * 768], bf16, tag="wT")
    woT_sb = consts.tile([128, 2 * C], bf16, tag="woT")
    cos_sb = consts.tile([128, T], bf16, tag="cos")
    sin_sb = consts.tile([128, T], bf16, tag="sin")
    amask_sb = consts.tile([128, 256], bf16, tag="amask")
    id_sb = consts.tile([128, 128], bf16, tag="ident")

    # (wT kc, xT kc) pairs alternate across the SP and ACT DMA queues so
    # the first contraction chunks land ~3us in and the rest stream behind.
    def _ld_w(eng, kc):
        eng.dma_start(out=wT_sb[:, kc * 768:(kc + 1) * 768],
                      in_=wT[kc * 128:(kc + 1) * 128, :])

    def _ld_x(eng, kc, th):
        eng.dma_start(
            out=xT_sb[:, kc * T + th * 1024:kc * T + (th + 1) * 1024],
            in_=xT[kc * 128:(kc + 1) * 128, th * 1024:(th + 1) * 1024])

    for kc in range(0, 8, 2):
        _ld_w(nc.sync, kc)
        _ld_x(nc.sync, kc, 0)
        _ld_w(nc.scalar, kc + 1)
        _ld_x(nc.scalar, kc + 1, 0)
    nc.scalar.dma_start(out=cos_sb, in_=cos4)
    nc.scalar.dma_start(out=sin_sb, in_=sin4)
    for kc in range(8):
        _ld_x(nc.scalar, kc, 1)
    nc.scalar.dma_start(out=amask_sb, in_=amask01)
    nc.scalar.dma_start(out=id_sb, in_=ident)
    nc.scalar.dma_start(
        out=woT_sb.rearrange("p (kc e) -> p kc e", kc=2),
        in_=woT.rearrange("(kc p) e -> p kc e", p=128),
    )

    # ---- persistent intermediates ----
    # rotated q/k blocks [q_x1, q_x2, k_x1, k_x2], each [128=(4h x 32d), T]
    rot = [stage.tile([128, T], bf16, tag=f"rot{i}", name=f"rot{i}")
           for i in range(4)]
    qhT = stage.tile([64, HEADS_PER_CORE * T], bf16, tag="qhT")
    khT = stage.tile([64, HEADS_PER_CORE * T], bf16, tag="khT")
    # v in [k-token-part, (kt, head, 65)] layout; col 64 of each head = ones
    v_sb = stage.tile([128, QT * VROW], bf16, tag="v")
    nc.gpsimd.memset(
        v_sb.rearrange("p (g c) -> p g c", c=VW)[:, :, DH:DH + 1], 1.0)

    pres = {}  # split -> [pre tiles]

    def qkv_mm(split):
        """QKV projection matmuls + PSUM->SBUF casts for one token slice."""
        ptiles = []
        for blk in range(4):  # q_x1 q_x2 k_x1 k_x2
            acc = pmm.tile([128, 512], f32, tag="mm")
            for kc in range(8):
                nc.tensor.matmul(
                    acc,
                    lhsT=wT_sb[:, kc * 768 + blk * 128:kc * 768 + (blk + 1) * 128],
                    rhs=xT_sb[:, kc * T + split * 512:kc * T + (split + 1) * 512],
                    start=(kc == 0),
                    stop=(kc == 7),
                )
            pblk = pre.tile([128, 512], bf16, tag="pre", name=f"pre{split}{blk}")
            nc.scalar.copy(pblk, acc)
            ptiles.append(pblk)
        pres[split] = ptiles

    def rope(split, pair, eng):
        """rot1 = x1*cos - x2*sin ; rot2 = x2*cos + x1*sin for one pair."""
        tsl = slice(split * 512, (split + 1) * 512)
        x1, x2 = pres[split][2 * pair], pres[split][2 * pair + 1]
        r1, r2 = rot[2 * pair][:, tsl], rot[2 * pair + 1][:, tsl]
        t1 = tmp.tile([128, 512], bf16, tag="t1")
        t2 = tmp.tile([128, 512], bf16, tag="t2")
        t3 = tmp.tile([128, 512], bf16, tag="t3")
        t4 = tmp.tile([128, 512], bf16, tag="t4")
        eng.tensor_mul(t1, x1, cos_sb[:, tsl])
        eng.tensor_mul(t2, x2, sin_sb[:, tsl])
        eng.tensor_sub(r1, t1, t2)
        eng.tensor_mul(t3, x2, cos_sb[:, tsl])
        eng.tensor_mul(t4, x1, sin_sb[:, tsl])
        eng.tensor_add(r2, t3, t4)

    def repack(t0, tlen):
        """Repack a token range of rot into head-major qhT/khT."""
        tsl = slice(t0, t0 + tlen)
        for hl in range(HEADS_PER_CORE):
            d0 = hl * T + t0
            for half in range(2):
                nc.sync.dma_start(
                    out=qhT[half * 32:(half + 1) * 32, d0:d0 + tlen],
                    in_=rot[half][hl * 32:(hl + 1) * 32, tsl],
                )
                nc.sync.dma_start(
                    out=khT[half * 32:(half + 1) * 32, d0:d0 + tlen],
                    in_=rot[2 + half][hl * 32:(hl + 1) * 32, tsl],
                )

    # ---- software-pipelined attention ----
    st = {}  # qt -> {p:{hl: tile}, osb:, asb:}

    def wincfg(qt):
        nkt = min(qt + 1, 3)
        return nkt, max(qt - 2, 0)

    def emit_v(qt):
        """V tile for qt in [k-part, (head, 65)] layout (ones col fused)."""
        acc = pmm.tile([128, FQ], f32, tag="mm")
        for kc in range(8):
            nc.tensor.matmul(
                acc,
                lhsT=xT_sb[:, kc * T + qt * 128:kc * T + (qt + 1) * 128],
                rhs=wT_sb[:, kc * 768 + 512:kc * 768 + 768],
                start=(kc == 0),
                stop=(kc == 7),
            )
        nc.scalar.copy(
            v_sb[:, qt * VROW:(qt + 1) * VROW]
            .rearrange("p (h c) -> p h c", h=HEADS_PER_CORE)[:, :, 0:DH],
            acc.rearrange("p (h d) -> p h d", h=HEADS_PER_CORE),
        )

    def emit_scores(qt, heads):
        """Transposed scores + exp + band mask for a pair of heads."""
        nkt, kt0 = wincfg(qt)
        w = 128 * nkt
        ss = st.setdefault(qt, {"p": {}})
        for hl in heads:
            s = ps.tile([128, 384], f32, tag="s")
            for a in range(nkt):
                kt = kt0 + a
                nc.tensor.matmul(
                    s[:, a * 128:(a + 1) * 128],
                    lhsT=khT[:, hl * T + kt * 128:hl * T + (kt + 1) * 128],
                    rhs=qhT[:, hl * T + qt * 128:hl * T + (qt + 1) * 128],
                    start=True,
                    stop=True,
                )
            p = work.tile([128, 384], bf16, tag="p")
            nc.scalar.activation(p[:, :w], s[:, :w], Exp)
            if qt >= 2:  # zero both triangle blocks in one strided bf16 op
                pv = p.rearrange("p (b w) -> p b w", b=3)[:, 0::2, :]
                mv = amask_sb.rearrange("p (b w) -> p b w", b=2)
                nc.vector.tensor_mul(pv, pv, mv)
            else:  # only the diagonal block needs masking
                seg = p[:, (nkt - 1) * 128:nkt * 128]
                nc.vector.tensor_mul(seg, seg, amask_sb[:, 128:256])
            ss["p"][hl] = p

    def emit_ot(qt):
        """P^T @ [V|1] per head, then per-partition normalization."""
        nkt, kt0 = wincfg(qt)
        ss = st[qt]
        osb = osbp.tile([128, FQ], bf16, tag="osb")
        ss["osb"] = osb
        for hl in range(HEADS_PER_CORE):
            p = ss["p"][hl]
            o = po.tile([128, VW], f32, tag="o")
            for a in range(nkt):
                kt = kt0 + a
                nc.tensor.matmul(
                    o,
                    lhsT=p[:, a * 128:(a + 1) * 128],
                    rhs=v_sb[:, kt * VROW + hl * VW:kt * VROW + (hl + 1) * VW],
                    start=(a == 0),
                    stop=(a == nkt - 1),
                )
            rc = small.tile([128, 1], f32, tag="rc")
            nc.vector.reciprocal(rc, o[:, DH:DH + 1])
            nc.vector.tensor_scalar_mul(
                osb[:, hl * DH:(hl + 1) * DH], o[:, 0:DH], rc)

    def emit_tp(qt):
        """PE-transpose the attention output to [feature, token]."""
        if qt < 0:
            return
        ss = st[qt]
        t2 = ps.tile([128, FQ], bf16, tag="s", name="t2")
        for c in range(2):
            nc.tensor.transpose(
                t2[:, c * 128:(c + 1) * 128],
                ss["osb"][:, c * 128:(c + 1) * 128], id_sb)
        asb = asbp.tile([128, FQ], bf16, tag="asb")
        nc.vector.tensor_copy(asb, t2)
        ss["asb"] = asb

    def emit_outproj(qt):
        """Out-proj for qt, staged through SBUF (cast to bf16) and stored."""
        if qt < 0:
            return
        asb = st[qt]["asb"]
        ysb = ysbp.tile([128, C], bf16, tag="ysb")
        for nh in range(2):
            acc = pout.tile([128, 512], f32, tag="yp")
            for kc in range(2):
                nc.tensor.matmul(
                    acc,
                    lhsT=asb[:, kc * 128:(kc + 1) * 128],
                    rhs=woT_sb[:, kc * C + nh * 512:kc * C + (nh + 1) * 512],
                    start=(kc == 0),
                    stop=(kc == 1),
                )
            if nh == 0:
                nc.scalar.copy(ysb[:, 0:512], acc)
            else:
                nc.vector.tensor_copy(ysb[:, 512:1024], acc)
        nc.sync.dma_start(out=y[qt * 128:(qt + 1) * 128, :], in_=ysb)
        del st[qt]

    vq = [4]  # next V tile to emit (prologue covers 0..3)

    def attn_iter(qt):
        emit_tp(qt - 1)
        emit_scores(qt, (0, 1))
        if vq[0] < QT:
            emit_v(vq[0])
            vq[0] += 1
        emit_scores(qt, (2, 3))
        emit_ot(qt)
        emit_outproj(qt - 1)

    # ---- prologue: projections for token half 0, RoPE on idle DVE,
    # V tiles 0-3 as PE filler while the head-major repack lands ----
    qkv_mm(0)
    rope(0, 0, nc.vector)
    rope(0, 1, nc.vector)
    qkv_mm(1)
    rope(1, 0, nc.vector)
    rope(1, 1, nc.vector)
    repack(0, 1024)
    for qt in range(4):
        emit_v(qt)

    # ---- attention pipeline, with split 2/3 projections interleaved ----
    # steady-state RoPE runs on the otherwise-idle GpSimd engine; each
    # split's repack is issued a few iterations before its consumers
    attn_iter(0)
    qkv_mm(2)
    attn_iter(1)
    rope(2, 0, nc.gpsimd)
    rope(2, 1, nc.gpsimd)
    qkv_mm(3)
    attn_iter(2)
    rope(3, 0, nc.gpsimd)
    rope(3, 1, nc.gpsimd)
    attn_iter(3)
    repack(1024, 512)
    attn_iter(4)
    attn_iter(5)
    repack(1536, 512)
    for qt in range(6, QT):
        attn_iter(qt)
    emit_tp(QT - 1)
    emit_outproj(QT - 1)


def _build_program():
    import concourse.tile as tile
    from concourse import bacc, mybir

    bf16 = mybir.dt.bfloat16

    nc = bacc.Bacc("TRN2", target_bir_lowering=False, debug=False,
                   num_devices=N_CORES)
    aps = {
        "xT": nc.dram_tensor("xT", [C, T], bf16, kind="ExternalInput").ap(),
        "wT": nc.dram_tensor("wT", [C, 768], bf16, kind="ExternalInput").ap(),
        "woT": nc.dram_tensor("woT", [FQ, C], bf16, kind="ExternalInput").ap(),
        "cos4": nc.dram_tensor("cos4", [128, T], bf16, kind="ExternalInput").ap(),
        "sin4": nc.dram_tensor("sin4", [128, T], bf16, kind="ExternalInput").ap(),
        "amask01": nc.dram_tensor("amask01", [128, 256], bf16, kind="ExternalInput").ap(),
        "ident": nc.dram_tensor("ident", [128, 128], bf16, kind="ExternalInput").ap(),
        "y": nc.dram_tensor("y", [T, C], bf16, kind="ExternalOutput").ap(),
    }
    from contextlib import ExitStack

    with tile.TileContext(nc) as tc, ExitStack() as ctx:
        _emit(nc, tc, aps, ctx)
    nc.compile()
    return nc


def _get_program():
    global _PROGRAM
    if _PROGRAM is None:
        _PROGRAM = _build_program()
    return _PROGRAM


def _host_inputs(x, w_qkv, w_out):
    import ml_dtypes

    bf16 = ml_dtypes.bfloat16
    x = np.asarray(x, np.float32)
    w_qkv = np.asarray(w_qkv, np.float32)
    w_out = np.asarray(w_out, np.float32)

    wq, wk, wv = w_qkv[0:C], w_qkv[C:2 * C], w_qkv[2 * C:3 * C]
    scale = 1.0 / math.sqrt(DH)

    # RoPE tables (transposed, tiled over the 4 heads of a block)
    inv_freq = 1.0 / (10000.0 ** (np.arange(0, DH, 2, dtype=np.float32) / DH))
    freqs = np.outer(np.arange(T, dtype=np.float32), inv_freq)  # [T, 32]
    cos4 = np.ascontiguousarray(np.tile(np.cos(freqs).T, (4, 1))).astype(bf16)
    sin4 = np.ascontiguousarray(np.tile(np.sin(freqs).T, (4, 1))).astype(bf16)

    # multiplicative 0/1 band masks for TRANSPOSED probabilities pT[k, q]:
    # [block kt=qt-2: allowed qq < kk | block kt=qt: allowed qq >= kk]
    i = np.arange(128)[:, None]  # kk (partitions)
    c = np.arange(128)[None, :]  # qq (free)
    m_first = (c < i).astype(np.float32)
    m_last = (c >= i).astype(np.float32)
    amask01 = np.ascontiguousarray(
        np.concatenate([m_first, m_last], axis=1)).astype(bf16)
    ident = np.eye(128, dtype=np.float32).astype(bf16)

    xT = [np.ascontiguousarray(x[b].T).astype(bf16) for b in range(B)]

    in_maps = []
    for core in range(N_CORES):
        b, g = divmod(core, 4)
        hs = range(4 * g, 4 * g + 4)
        rows = []
        for half in range(2):  # q_x1, q_x2
            rows.append(np.concatenate(
                [wq[h * DH + 32 * half:h * DH + 32 * half + 32] for h in hs]) * scale)
        for half in range(2):  # k_x1, k_x2
            rows.append(np.concatenate(
                [wk[h * DH + 32 * half:h * DH + 32 * half + 32] for h in hs]))
        rows.append(wv[g * FQ:(g + 1) * FQ])
        wmat = np.concatenate(rows)  # [768, C]
        wT = np.ascontiguousarray(wmat.T).astype(bf16)
        woT = np.ascontiguousarray(w_out[:, g * FQ:(g + 1) * FQ].T).astype(bf16)
        in_maps.append({
            "xT": xT[b], "wT": wT, "woT": woT,
            "cos4": cos4, "sin4": sin4, "amask01": amask01, "ident": ident,
        })
    return in_maps


def kernel(x, w_qkv, w_out, _trace=False):
    from concourse import bass_utils

    nc = _get_program()
    in_maps = _host_inputs(x, w_qkv, w_out)
    res = bass_utils.run_bass_kernel_spmd(
        nc, in_maps, core_ids=list(range(N_CORES)), trace=_trace,
    )
    parts = [np.asarray(res.results[core]["y"], dtype=np.float32)
             for core in range(N_CORES)]
    out = np.stack([
        parts[0] + parts[1] + parts[2] + parts[3],
        parts[4] + parts[5] + parts[6] + parts[7],
    ])
    if _trace:
        return out, res
    return out


# revision 16
# speedup vs baseline: 1.0956x; 1.0956x over previous
"""Sliding-window causal self-attention (B=2, T=2048, C=1024, H=16, Dh=64,
window=256) + QKV/out projections, sharded over 8 NeuronCores as
data-parallel over B (2) x tensor-parallel over head groups (4 heads/core).

Layout strategy ("sT scheme"): scores are computed TRANSPOSED
(sT[k, q] = khT^T @ qhT) so the exp() activation writes P^T straight to
SBUF. The band mask is a post-exp 0/1 multiply on bf16 SBUF data. Row
sums come free from a ones-column appended to each head's V tile
(oT_ext[:, 64] = rowsum, q on partitions), so normalization is a
per-partition scalar multiply. The attention output is transposed back
with PE identity matmuls right before the out-proj.

Scheduling: input chunks are issued pairwise on the SP and ACT DMA
queues so the first QKV matmul starts ~3us in. The emission is
software-pipelined per query tile so the PE always has independent work
while the exp chain drains:
  [sT h0,h1] [transpose qt-1] [V qt+1] [sT h2,h3] [oT h0..3] [outproj qt-1]
QKV token-splits are emitted several iterations before their attention
consumers; RoPE runs on DVE (q side) and the otherwise-idle GpSimd
engine (k side) so the head-major repack is ready with slack.
"""

import math

import numpy as np

B = 2
T = 2048
C = 1024
H = 16
DH = 64
WINDOW = 256
HEADS_PER_CORE = 4
N_CORES = 8
QT = T // 128  # 16 query tiles of 128
FQ = HEADS_PER_CORE * DH  # 256 local features
VW = DH + 1  # per-head v columns incl the fused ones column
VROW = HEADS_PER_CORE * VW  # 260 v columns per key tile

_PROGRAM = None  # compile once per process


def _emit(nc, tc, aps, ctx):
    from concourse import mybir

    f32 = mybir.dt.float32
    bf16 = mybir.dt.bfloat16
    Exp = mybir.ActivationFunctionType.Exp

    xT, wT, woT, cos4, sin4, amask01, ident, y = (
        aps["xT"], aps["wT"], aps["woT"], aps["cos4"], aps["sin4"],
        aps["amask01"], aps["ident"], aps["y"],
    )

    consts = ctx.enter_context(tc.tile_pool(name="consts", bufs=1))
    stage = ctx.enter_context(tc.tile_pool(name="stage", bufs=1))
    pre = ctx.enter_context(tc.tile_pool(name="pre", bufs=8))
    tmp = ctx.enter_context(tc.tile_pool(name="tmp", bufs=2))
    work = ctx.enter_context(tc.tile_pool(name="work", bufs=6))
    osbp = ctx.enter_context(tc.tile_pool(name="osbp", bufs=2))
    asbp = ctx.enter_context(tc.tile_pool(name="asbp", bufs=2))
    ysbp = ctx.enter_context(tc.tile_pool(name="ysbp", bufs=3))
    small = ctx.enter_context(tc.tile_pool(name="small", bufs=4))
    pmm = ctx.enter_context(tc.tile_pool(name="pmm", bufs=2, space="PSUM"))
    pout = ctx.enter_context(tc.tile_pool(name="pout", bufs=2, space="PSUM"))
    ps = ctx.enter_context(tc.tile_pool(name="ps", bufs=2, space="PSUM"))
    po = ctx.enter_context(tc.tile_pool(name="po", bufs=2, space="PSUM"))

    # ---- resident inputs ----
    xT_sb = consts.tile([128, 8 * T], bf16, tag="xT")  # [C-part, (kc t)]
    wT_sb = consts.tile([128, 8 * 768], bf16, tag="wT")
    woT_sb = consts.tile([128, 2 * C], bf16, tag="woT")
    cos_sb = consts.tile([128, T], bf16, tag="cos")
    sin_sb = consts.tile([128, T], bf16, tag="sin")
    amask_sb = consts.tile([128, 256], bf16, tag="amask")
    id_sb = consts.tile([128, 128], bf16, tag="ident")

    # (wT kc, xT kc) pairs alternate across the SP and ACT DMA queues so
    # the first contraction chunks land ~3us in and the rest stream behind.
    def _ld_w(eng, kc):
        eng.dma_start(out=wT_sb[:, kc * 768:(kc + 1) * 768],
                      in_=wT[kc * 128:(kc + 1) * 128, :])

    def _ld_x(eng, kc, th):
        eng.dma_start(
            out=xT_sb[:, kc * T + th * 1024:kc * T + (th + 1) * 1024],
            in_=xT[kc * 128:(kc + 1) * 128, th * 1024:(th + 1) * 1024])

    for kc in range(0, 8, 2):
        _ld_w(nc.sync, kc)
        _ld_x(nc.sync, kc, 0)
        _ld_w(nc.scalar, kc + 1)
        _ld_x(nc.scalar, kc + 1, 0)
    nc.scalar.dma_start(out=cos_sb, in_=cos4)
    nc.scalar.dma_start(out=sin_sb, in_=sin4)
    for kc in range(8):
        _ld_x(nc.scalar, kc, 1)
    nc.scalar.dma_start(out=amask_sb, in_=amask01)
    nc.scalar.dma_start(out=id_sb, in_=ident)
    nc.scalar.dma_start(
        out=woT_sb.rearrange("p (kc e) -> p kc e", kc=2),
        in_=woT.rearrange("(kc p) e -> p kc e", p=128),
    )

    # ---- persistent intermediates ----
    # rotated q/k blocks [q_x1, q_x2, k_x1, k_x2], each [128=(4h x 32d), T]
    rot = [stage.tile([128, T], bf16, tag=f"rot{i}", name=f"rot{i}")
           for i in range(4)]
    qhT = stage.tile([64, HEADS_PER_CORE * T], bf16, tag="qhT")
    khT = stage.tile([64, HEADS_PER_CORE * T], bf16, tag="khT")
    # v in [k-token-part, (kt, head, 65)] layout; col 64 of each head = ones
    v_sb = stage.tile([128, QT * VROW], bf16, tag="v")
    nc.gpsimd.memset(
        v_sb.rearrange("p (g c) -> p g c", c=VW)[:, :, DH:DH + 1], 1.0)

    pres = {}  # split -> [pre tiles]

    def qkv_mm(split):
        """QKV projection matmuls + PSUM->SBUF casts for one token slice."""
        ptiles = []
        for blk in range(4):  # q_x1 q_x2 k_x1 k_x2
            acc = pmm.tile([128, 512], f32, tag="mm")
            for kc in range(8):
                nc.tensor.matmul(
                    acc,
                    lhsT=wT_sb[:, kc * 768 + blk * 128:kc * 768 + (blk + 1) * 128],
                    rhs=xT_sb[:, kc * T + split * 512:kc * T + (split + 1) * 512],
                    start=(kc == 0),
                    stop=(kc == 7),
                )
            pblk = pre.tile([128, 512], bf16, tag="pre", name=f"pre{split}{blk}")
            nc.scalar.copy(pblk, acc)
            ptiles.append(pblk)
        pres[split] = ptiles

    def rope(split, pair, eng):
        """rot1 = x1*cos - x2*sin ; rot2 = x2*cos + x1*sin for one pair."""
        tsl = slice(split * 512, (split + 1) * 512)
        x1, x2 = pres[split][2 * pair], pres[split][2 * pair + 1]
        r1, r2 = rot[2 * pair][:, tsl], rot[2 * pair + 1][:, tsl]
        t1 = tmp.tile([128, 512], bf16, tag="t1")
        t2 = tmp.tile([128, 512], bf16, tag="t2")
        t3 = tmp.tile([128, 512], bf16, tag="t3")
        t4 = tmp.tile([128, 512], bf16, tag="t4")
        eng.tensor_mul(t1, x1, cos_sb[:, tsl])
        eng.tensor_mul(t2, x2, sin_sb[:, tsl])
        eng.tensor_sub(r1, t1, t2)
        eng.tensor_mul(t3, x2, cos_sb[:, tsl])
        eng.tensor_mul(t4, x1, sin_sb[:, tsl])
        eng.tensor_add(r2, t3, t4)

    def repack(t0, tlen):
        """Repack a token range of rot into head-major qhT/khT."""
        tsl = slice(t0, t0 + tlen)
        for hl in range(HEADS_PER_CORE):
            d0 = hl * T + t0
            for half in range(2):
                nc.sync.dma_start(
                    out=qhT[half * 32:(half + 1) * 32, d0:d0 + tlen],
                    in_=rot[half][hl * 32:(hl + 1) * 32, tsl],
                )
                nc.sync.dma_start(
                    out=khT[half * 32:(half + 1) * 32, d0:d0 + tlen],
                    in_=rot[2 + half][hl * 32:(hl + 1) * 32, tsl],
                )

    # ---- software-pipelined attention ----
    st = {}  # qt -> {p:{hl: tile}, osb:, asb:}

    def wincfg(qt):
        nkt = min(qt + 1, 3)
        return nkt, max(qt - 2, 0)

    def emit_v(qt):
        """V tile for qt in [k-part, (head, 65)] layout (ones col fused)."""
        acc = pmm.tile([128, FQ], f32, tag="mm")
        for kc in range(8):
            nc.tensor.matmul(
                acc,
                lhsT=xT_sb[:, kc * T + qt * 128:kc * T + (qt + 1) * 128],
                rhs=wT_sb[:, kc * 768 + 512:kc * 768 + 768],
                start=(kc == 0),
                stop=(kc == 7),
            )
        nc.scalar.copy(
            v_sb[:, qt * VROW:(qt + 1) * VROW]
            .rearrange("p (h c) -> p h c", h=HEADS_PER_CORE)[:, :, 0:DH],
            acc.rearrange("p (h d) -> p h d", h=HEADS_PER_CORE),
        )

    def emit_scores(qt, heads):
        """Transposed scores + exp + band mask for a pair of heads."""
        nkt, kt0 = wincfg(qt)
        w = 128 * nkt
        ss = st.setdefault(qt, {"p": {}})
        for hl in heads:
            s = ps.tile([128, 384], f32, tag="s")
            for a in range(nkt):
                kt = kt0 + a
                nc.tensor.matmul(
                    s[:, a * 128:(a + 1) * 128],
                    lhsT=khT[:, hl * T + kt * 128:hl * T + (kt + 1) * 128],
                    rhs=qhT[:, hl * T + qt * 128:hl * T + (qt + 1) * 128],
                    start=True,
                    stop=True,
                )
            p = work.tile([128, 384], bf16, tag="p")
            nc.scalar.activation(p[:, :w], s[:, :w], Exp)
            if qt >= 2:  # zero both triangle blocks in one strided bf16 op
                pv = p.rearrange("p (b w) -> p b w", b=3)[:, 0::2, :]
                mv = amask_sb.rearrange("p (b w) -> p b w", b=2)
                nc.vector.tensor_mul(pv, pv, mv)
            else:  # only the diagonal block needs masking
                seg = p[:, (nkt - 1) * 128:nkt * 128]
                nc.vector.tensor_mul(seg, seg, amask_sb[:, 128:256])
            ss["p"][hl] = p

    def emit_ot(qt):
        """P^T @ [V|1] per head, then per-partition normalization."""
        nkt, kt0 = wincfg(qt)
        ss = st[qt]
        osb = osbp.tile([128, FQ], bf16, tag="osb")
        ss["osb"] = osb
        for hl in range(HEADS_PER_CORE):
            p = ss["p"][hl]
            o = po.tile([128, VW], f32, tag="o")
            for a in range(nkt):
                kt = kt0 + a
                nc.tensor.matmul(
                    o,
                    lhsT=p[:, a * 128:(a + 1) * 128],
                    rhs=v_sb[:, kt * VROW + hl * VW:kt * VROW + (hl + 1) * VW],
                    start=(a == 0),
                    stop=(a == nkt - 1),
                )
            rc = small.tile([128, 1], f32, tag="rc")
            nc.vector.reciprocal(rc, o[:, DH:DH + 1])
            nc.vector.tensor_scalar_mul(
                osb[:, hl * DH:(hl + 1) * DH], o[:, 0:DH], rc)

    def emit_tp(qt):
        """PE-transpose the attention output to [feature, token]."""
        if qt < 0:
            return
        ss = st[qt]
        t2 = ps.tile([128, FQ], bf16, tag="s", name="t2")
        for c in range(2):
            nc.tensor.transpose(
                t2[:, c * 128:(c + 1) * 128],
                ss["osb"][:, c * 128:(c + 1) * 128], id_sb)
        asb = asbp.tile([128, FQ], bf16, tag="asb")
        nc.vector.tensor_copy(asb, t2)
        ss["asb"] = asb

    def emit_outproj(qt):
        """Out-proj for qt, staged through SBUF (cast to bf16) and stored."""
        if qt < 0:
            return
        asb = st[qt]["asb"]
        ysb = ysbp.tile([128, C], bf16, tag="ysb")
        for nh in range(2):
            acc = pout.tile([128, 512], f32, tag="yp")
            for kc in range(2):
                nc.tensor.matmul(
                    acc,
                    lhsT=asb[:, kc * 128:(kc + 1) * 128],
                    rhs=woT_sb[:, kc * C + nh * 512:kc * C + (nh + 1) * 512],
                    start=(kc == 0),
                    stop=(kc == 1),
                )
            if nh == 0:
                nc.scalar.copy(ysb[:, 0:512], acc)
            else:
                nc.vector.tensor_copy(ysb[:, 512:1024], acc)
        nc.sync.dma_start(out=y[qt * 128:(qt + 1) * 128, :], in_=ysb)
        del st[qt]

    def attn_iter(qt):
        emit_scores(qt, (0, 1))
        emit_tp(qt - 1)
        if qt + 1 < QT:
            emit_v(qt + 1)
        emit_scores(qt, (2, 3))
        emit_ot(qt)
        emit_outproj(qt - 1)

    # ---- prologue: projections for token half 0, RoPE on idle DVE ----
    qkv_mm(0)
    rope(0, 0, nc.vector)
    rope(0, 1, nc.vector)
    qkv_mm(1)
    rope(1, 0, nc.vector)
    rope(1, 1, nc.vector)
    repack(0, 1024)
    emit_v(0)
    qkv_mm(2)  # keeps the PE busy while the repack lands

    # ---- attention pipeline, with split 2/3 projections interleaved ----
    # steady-state RoPE runs on the otherwise-idle GpSimd engine; each
    # split's repack is issued a few iterations before its consumers
    attn_iter(0)
    rope(2, 0, nc.gpsimd)
    rope(2, 1, nc.gpsimd)
    attn_iter(1)
    qkv_mm(3)
    attn_iter(2)
    rope(3, 0, nc.gpsimd)
    rope(3, 1, nc.gpsimd)
    attn_iter(3)
    repack(1024, 512)
    attn_iter(4)
    attn_iter(5)
    repack(1536, 512)
    for qt in range(6, QT):
        attn_iter(qt)
    emit_tp(QT - 1)
    emit_outproj(QT - 1)


def _build_program():
    import concourse.tile as tile
    from concourse import bacc, mybir

    bf16 = mybir.dt.bfloat16

    nc = bacc.Bacc("TRN2", target_bir_lowering=False, debug=False,
                   num_devices=N_CORES)
    aps = {
        "xT": nc.dram_tensor("xT", [C, T], bf16, kind="ExternalInput").ap(),
        "wT": nc.dram_tensor("wT", [C, 768], bf16, kind="ExternalInput").ap(),
        "woT": nc.dram_tensor("woT", [FQ, C], bf16, kind="ExternalInput").ap(),
        "cos4": nc.dram_tensor("cos4", [128, T], bf16, kind="ExternalInput").ap(),
        "sin4": nc.dram_tensor("sin4", [128, T], bf16, kind="ExternalInput").ap(),
        "amask01": nc.dram_tensor("amask01", [128, 256], bf16, kind="ExternalInput").ap(),
        "ident": nc.dram_tensor("ident", [128, 128], bf16, kind="ExternalInput").ap(),
        "y": nc.dram_tensor("y", [T, C], bf16, kind="ExternalOutput").ap(),
    }
    from contextlib import ExitStack

    with tile.TileContext(nc) as tc, ExitStack() as ctx:
        _emit(nc, tc, aps, ctx)
    nc.compile()
    return nc


def _get_program():
    global _PROGRAM
    if _PROGRAM is None:
        _PROGRAM = _build_program()
    return _PROGRAM


def _host_inputs(x, w_qkv, w_out):
    import ml_dtypes

    bf16 = ml_dtypes.bfloat16
    x = np.asarray(x, np.float32)
    w_qkv = np.asarray(w_qkv, np.float32)
    w_out = np.asarray(w_out, np.float32)

    wq, wk, wv = w_qkv[0:C], w_qkv[C:2 * C], w_qkv[2 * C:3 * C]
    scale = 1.0 / math.sqrt(DH)

    # RoPE tables (transposed, tiled over the 4 heads of a block)
    inv_freq = 1.0 / (10000.0 ** (np.arange(0, DH, 2, dtype=np.float32) / DH))
    freqs = np.outer(np.arange(T, dtype=np.float32), inv_freq)  # [T, 32]
    cos4 = np.ascontiguousarray(np.tile(np.cos(freqs).T, (4, 1))).astype(bf16)
    sin4 = np.ascontiguousarray(np.tile(np.sin(freqs).T, (4, 1))).astype(bf16)

    # multiplicative 0/1 band masks for TRANSPOSED probabilities pT[k, q]:
    # [block kt=qt-2: allowed qq < kk | block kt=qt: allowed qq >= kk]
    i = np.arange(128)[:, None]  # kk (partitions)
    c = np.arange(128)[None, :]  # qq (free)
    m_first = (c < i).astype(np.float32)
    m_last = (c >= i).astype(np.float32)
    amask01 = np.ascontiguousarray(
        np.concatenate([m_first, m_last], axis=1)).astype(bf16)
    ident = np.eye(128, dtype=np.float32).astype(bf16)

    xT = [np.ascontiguousarray(x[b].T).astype(bf16) for b in range(B)]

    in_maps = []
    for core in range(N_CORES):
        b, g = divmod(core, 4)
        hs = range(4 * g, 4 * g + 4)
        rows = []
        for half in range(2):  # q_x1, q_x2
            rows.append(np.concatenate(
                [wq[h * DH + 32 * half:h * DH + 32 * half + 32] for h in hs]) * scale)
        for half in range(2):  # k_x1, k_x2
            rows.append(np.concatenate(
                [wk[h * DH + 32 * half:h * DH + 32 * half + 32] for h in hs]))
        rows.append(wv[g * FQ:(g + 1) * FQ])
        wmat = np.concatenate(rows)  # [768, C]
        wT = np.ascontiguousarray(wmat.T).astype(bf16)
        woT = np.ascontiguousarray(w_out[:, g * FQ:(g + 1) * FQ].T).astype(bf16)
        in_maps.append({
            "xT": xT[b], "wT": wT, "woT": woT,
            "cos4": cos4, "sin4": sin4, "amask01": amask01, "ident": ident,
        })
    return in_maps


def kernel(x, w_qkv, w_out, _trace=False):
    from concourse import bass_utils

    nc = _get_program()
    in_maps = _host_inputs(x, w_qkv, w_out)
    res = bass_utils.run_bass_kernel_spmd(
        nc, in_maps, core_ids=list(range(N_CORES)), trace=_trace,
    )
    parts = [np.asarray(res.results[core]["y"], dtype=np.float32)
             for core in range(N_CORES)]
    out = np.stack([
        parts[0] + parts[1] + parts[2] + parts[3],
        parts[4] + parts[5] + parts[6] + parts[7],
    ])
    if _trace:
        return out, res
    return out


# revision 19
# speedup vs baseline: 1.1357x; 1.0366x over previous
"""Sliding-window causal self-attention (B=2, T=2048, C=1024, H=16, Dh=64,
window=256) + QKV/out projections, sharded over 8 NeuronCores as
data-parallel over B (2) x tensor-parallel over head groups (4 heads/core).

Layout strategy ("sT scheme"): scores are computed TRANSPOSED
(sT[k, q] = khT^T @ qhT) so the exp() activation writes P^T straight to
SBUF. The band mask is a post-exp 0/1 multiply on bf16 SBUF data. Row
sums come free from a ones-column appended to each head's V tile
(oT_ext[:, 64] = rowsum, q on partitions), so normalization is a
per-partition scalar multiply. The attention output is transposed back
with PE identity matmuls right before the out-proj.

Scheduling: input chunks are issued pairwise on the SP and ACT DMA
queues so the first QKV matmul starts ~3us in. The emission is
software-pipelined per query tile so the PE always has independent work
while the exp chain drains:
  [sT h0,h1] [transpose qt-1] [V qt+1] [sT h2,h3] [oT h0..3] [outproj qt-1]
QKV token-splits are emitted several iterations before their attention
consumers; RoPE runs on DVE (q side) and the otherwise-idle GpSimd
engine (k side) so the head-major repack is ready with slack.
"""

import math

import numpy as np

B = 2
T = 2048
C = 1024
H = 16
DH = 64
WINDOW = 256
HEADS_PER_CORE = 4
N_CORES = 8
QT = T // 128  # 16 query tiles of 128
FQ = HEADS_PER_CORE * DH  # 256 local features
VW = DH + 1  # per-head v columns incl the fused ones column
VROW = HEADS_PER_CORE * VW  # 260 v columns per key tile

_PROGRAM = None  # compile once per process


def _emit(nc, tc, aps, ctx):
    from concourse import mybir

    f32 = mybir.dt.float32
    bf16 = mybir.dt.bfloat16
    Exp = mybir.ActivationFunctionType.Exp

    xT, wT, woT, cos4, sin4, amask01, ident, y = (
        aps["xT"], aps["wT"], aps["woT"], aps["cos4"], aps["sin4"],
        aps["amask01"], aps["ident"], aps["y"],
    )

    consts = ctx.enter_context(tc.tile_pool(name="consts", bufs=1))
    stage = ctx.enter_context(tc.tile_pool(name="stage", bufs=1))
    pre = ctx.enter_context(tc.tile_pool(name="pre", bufs=8))
    tmp = ctx.enter_context(tc.tile_pool(name="tmp", bufs=2))
    work = ctx.enter_context(tc.tile_pool(name="work", bufs=6))
    osbp = ctx.enter_context(tc.tile_pool(name="osbp", bufs=2))
    asbp = ctx.enter_context(tc.tile_pool(name="asbp", bufs=2))
    ysbp = ctx.enter_context(tc.tile_pool(name="ysbp", bufs=3))
    small = ctx.enter_context(tc.tile_pool(name="small", bufs=4))
    pmm = ctx.enter_context(tc.tile_pool(name="pmm", bufs=2, space="PSUM"))
    pout = ctx.enter_context(tc.tile_pool(name="pout", bufs=2, space="PSUM"))
    ps = ctx.enter_context(tc.tile_pool(name="ps", bufs=2, space="PSUM"))
    po = ctx.enter_context(tc.tile_pool(name="po", bufs=2, space="PSUM"))

    # ---- resident inputs ----
    xT_sb = consts.tile([128, 8 * T], bf16, tag="xT")  # [C-part, (kc t)]
    wT_sb = consts.tile([128, 8 * 768], bf16, tag="wT")
    woT_sb = consts.tile([128, 2 * C], bf16, tag="woT")
    cos_sb = consts.tile([128, T], bf16, tag="cos")
    sin_sb = consts.tile([128, T], bf16, tag="sin")
    amask_sb = consts.tile([128, 256], bf16, tag="amask")
    id_sb = consts.tile([128, 128], bf16, tag="ident")

    # (wT kc, xT kc) pairs alternate across the SP and ACT DMA queues so
    # the first contraction chunks land ~3us in and the rest stream behind.
    def _ld_w(eng, kc):
        eng.dma_start(out=wT_sb[:, kc * 768:(kc + 1) * 768],
                      in_=wT[kc * 128:(kc + 1) * 128, :])

    def _ld_x(eng, kc, th):
        eng.dma_start(
            out=xT_sb[:, kc * T + th * 1024:kc * T + (th + 1) * 1024],
            in_=xT[kc * 128:(kc + 1) * 128, th * 1024:(th + 1) * 1024])

    # ACT carries only 4 input pairs (its compute must start early);
    # SP carries the rest
    for kc in range(0, 8, 2):
        _ld_w(nc.sync, kc)
        _ld_x(nc.sync, kc, 0)
        _ld_w(nc.scalar, kc + 1)
        _ld_x(nc.scalar, kc + 1, 0)
    nc.sync.dma_start(out=cos_sb, in_=cos4)
    nc.sync.dma_start(out=sin_sb, in_=sin4)
    for kc in range(8):
        _ld_x(nc.sync, kc, 1)
    nc.sync.dma_start(out=amask_sb, in_=amask01)
    nc.sync.dma_start(out=id_sb, in_=ident)
    nc.sync.dma_start(
        out=woT_sb.rearrange("p (kc e) -> p kc e", kc=2),
        in_=woT.rearrange("(kc p) e -> p kc e", p=128),
    )

    # ---- persistent intermediates ----
    # rotated q/k blocks [q_x1, q_x2, k_x1, k_x2], each [128=(4h x 32d), T]
    rot = [stage.tile([128, T], bf16, tag=f"rot{i}", name=f"rot{i}")
           for i in range(4)]
    qhT = stage.tile([64, HEADS_PER_CORE * T], bf16, tag="qhT")
    khT = stage.tile([64, HEADS_PER_CORE * T], bf16, tag="khT")
    # v in [k-token-part, (kt, head, 65)] layout; col 64 of each head = ones
    v_sb = stage.tile([128, QT * VROW], bf16, tag="v")
    nc.gpsimd.memset(
        v_sb.rearrange("p (g c) -> p g c", c=VW)[:, :, DH:DH + 1], 1.0)

    pres = {}  # split -> [pre tiles]

    def qkv_half(split, pair, alt_pre=False):
        """QKV projection matmuls + PSUM->SBUF casts for the q or k blocks
        of one token slice. alt_pre routes one cast to DVE to avoid piling
        copies onto ACT ahead of latency-critical exps."""
        ptiles = pres.setdefault(split, [])
        for blk in (2 * pair, 2 * pair + 1):  # q_x1 q_x2 | k_x1 k_x2
            acc = pmm.tile([128, 512], f32, tag="mm")
            for kc in range(8):
                nc.tensor.matmul(
                    acc,
                    lhsT=wT_sb[:, kc * 768 + blk * 128:kc * 768 + (blk + 1) * 128],
                    rhs=xT_sb[:, kc * T + split * 512:kc * T + (split + 1) * 512],
                    start=(kc == 0),
                    stop=(kc == 7),
                )
            pblk = pre.tile([128, 512], bf16, tag="pre", name=f"pre{split}{blk}")
            if alt_pre and blk % 2 == 1:
                nc.vector.tensor_copy(pblk, acc)
            else:
                nc.scalar.copy(pblk, acc)
            ptiles.append(pblk)

    def rope(split, pair, eng):
        """rot1 = x1*cos - x2*sin ; rot2 = x2*cos + x1*sin for one pair."""
        tsl = slice(split * 512, (split + 1) * 512)
        x1, x2 = pres[split][2 * pair], pres[split][2 * pair + 1]
        r1, r2 = rot[2 * pair][:, tsl], rot[2 * pair + 1][:, tsl]
        t1 = tmp.tile([128, 512], bf16, tag="t1")
        t2 = tmp.tile([128, 512], bf16, tag="t2")
        t3 = tmp.tile([128, 512], bf16, tag="t3")
        t4 = tmp.tile([128, 512], bf16, tag="t4")
        eng.tensor_mul(t1, x1, cos_sb[:, tsl])
        eng.tensor_mul(t2, x2, sin_sb[:, tsl])
        eng.tensor_sub(r1, t1, t2)
        eng.tensor_mul(t3, x2, cos_sb[:, tsl])
        eng.tensor_mul(t4, x1, sin_sb[:, tsl])
        eng.tensor_add(r2, t3, t4)

    def repack(t0, tlen):
        """Repack a token range of rot into head-major qhT/khT."""
        tsl = slice(t0, t0 + tlen)
        for hl in range(HEADS_PER_CORE):
            d0 = hl * T + t0
            for half in range(2):
                nc.sync.dma_start(
                    out=qhT[half * 32:(half + 1) * 32, d0:d0 + tlen],
                    in_=rot[half][hl * 32:(hl + 1) * 32, tsl],
                )
                nc.sync.dma_start(
                    out=khT[half * 32:(half + 1) * 32, d0:d0 + tlen],
                    in_=rot[2 + half][hl * 32:(hl + 1) * 32, tsl],
                )

    # ---- software-pipelined attention ----
    st = {}  # qt -> {p:{hl: tile}, osb:, asb:}

    def wincfg(qt):
        nkt = min(qt + 1, 3)
        return nkt, max(qt - 2, 0)

    def emit_v(qt):
        """V tile for qt in [k-part, (head, 65)] layout (ones col fused)."""
        acc = pmm.tile([128, FQ], f32, tag="mm")
        for kc in range(8):
            nc.tensor.matmul(
                acc,
                lhsT=xT_sb[:, kc * T + qt * 128:kc * T + (qt + 1) * 128],
                rhs=wT_sb[:, kc * 768 + 512:kc * 768 + 768],
                start=(kc == 0),
                stop=(kc == 7),
            )
        nc.scalar.copy(
            v_sb[:, qt * VROW:(qt + 1) * VROW]
            .rearrange("p (h c) -> p h c", h=HEADS_PER_CORE)[:, :, 0:DH],
            acc.rearrange("p (h d) -> p h d", h=HEADS_PER_CORE),
        )

    def emit_scores(qt, heads):
        """Transposed scores + exp + band mask for a pair of heads."""
        nkt, kt0 = wincfg(qt)
        w = 128 * nkt
        ss = st.setdefault(qt, {"p": {}})
        for hl in heads:
            s = ps.tile([128, 384], f32, tag="s")
            for a in range(nkt):
                kt = kt0 + a
                nc.tensor.matmul(
                    s[:, a * 128:(a + 1) * 128],
                    lhsT=khT[:, hl * T + kt * 128:hl * T + (kt + 1) * 128],
                    rhs=qhT[:, hl * T + qt * 128:hl * T + (qt + 1) * 128],
                    start=True,
                    stop=True,
                )
            p = work.tile([128, 384], bf16, tag="p")
            nc.scalar.activation(p[:, :w], s[:, :w], Exp)
            if qt >= 2:  # zero both triangle blocks in one strided bf16 op
                pv = p.rearrange("p (b w) -> p b w", b=3)[:, 0::2, :]
                mv = amask_sb.rearrange("p (b w) -> p b w", b=2)
                nc.vector.tensor_mul(pv, pv, mv)
            else:  # only the diagonal block needs masking
                seg = p[:, (nkt - 1) * 128:nkt * 128]
                nc.vector.tensor_mul(seg, seg, amask_sb[:, 128:256])
            ss["p"][hl] = p

    def emit_ot(qt):
        """P^T @ [V|1] per head, then per-partition normalization."""
        nkt, kt0 = wincfg(qt)
        ss = st[qt]
        osb = osbp.tile([128, FQ], bf16, tag="osb")
        ss["osb"] = osb
        for hl in range(HEADS_PER_CORE):
            p = ss["p"][hl]
            o = po.tile([128, VW], f32, tag="o")
            for a in range(nkt):
                kt = kt0 + a
                nc.tensor.matmul(
                    o,
                    lhsT=p[:, a * 128:(a + 1) * 128],
                    rhs=v_sb[:, kt * VROW + hl * VW:kt * VROW + (hl + 1) * VW],
                    start=(a == 0),
                    stop=(a == nkt - 1),
                )
            rc = small.tile([128, 1], f32, tag="rc")
            nc.vector.reciprocal(rc, o[:, DH:DH + 1])
            nc.vector.tensor_scalar_mul(
                osb[:, hl * DH:(hl + 1) * DH], o[:, 0:DH], rc)

    def emit_tp(qt):
        """PE-transpose the attention output to [feature, token]."""
        if qt < 0:
            return
        ss = st[qt]
        t2 = ps.tile([128, FQ], bf16, tag="s", name="t2")
        for c in range(2):
            nc.tensor.transpose(
                t2[:, c * 128:(c + 1) * 128],
                ss["osb"][:, c * 128:(c + 1) * 128], id_sb)
        asb = asbp.tile([128, FQ], bf16, tag="asb")
        nc.vector.tensor_copy(asb, t2)
        ss["asb"] = asb

    def emit_outproj(qt):
        """Out-proj for qt, staged through SBUF (cast to bf16) and stored."""
        if qt < 0:
            return
        asb = st[qt]["asb"]
        ysb = ysbp.tile([128, C], bf16, tag="ysb")
        for nh in range(2):
            acc = pout.tile([128, 512], f32, tag="yp")
            for kc in range(2):
                nc.tensor.matmul(
                    acc,
                    lhsT=asb[:, kc * 128:(kc + 1) * 128],
                    rhs=woT_sb[:, kc * C + nh * 512:kc * C + (nh + 1) * 512],
                    start=(kc == 0),
                    stop=(kc == 1),
                )
            if nh == 0:
                nc.scalar.copy(ysb[:, 0:512], acc)
            else:
                nc.vector.tensor_copy(ysb[:, 512:1024], acc)
        nc.sync.dma_start(out=y[qt * 128:(qt + 1) * 128, :], in_=ysb)
        del st[qt]

    def attn_iter(qt):
        emit_scores(qt, (0, 1))
        emit_tp(qt - 1)
        if qt + 1 < QT:
            emit_v(qt + 1)
        emit_scores(qt, (2, 3))
        emit_ot(qt)
        emit_outproj(qt - 1)

    # ---- prologue: projections for token half 0, RoPE on idle DVE ----
    qkv_half(0, 0)
    qkv_half(0, 1)
    rope(0, 0, nc.vector)
    rope(0, 1, nc.vector)
    qkv_half(1, 0)
    qkv_half(1, 1)
    rope(1, 0, nc.vector)
    rope(1, 1, nc.vector)
    repack(0, 512)
    repack(512, 512)
    emit_v(0)
    qkv_half(2, 0)  # keeps the PE busy while the repack lands
    qkv_half(2, 1)

    # ---- attention pipeline, with split 2/3 projections interleaved ----
    # steady-state RoPE runs on the otherwise-idle GpSimd engine; each
    # split's repack is issued a few iterations before its consumers
    attn_iter(0)
    rope(2, 0, nc.gpsimd)
    rope(2, 1, nc.gpsimd)
    attn_iter(1)
    qkv_half(3, 0, alt_pre=True)
    rope(3, 0, nc.gpsimd)
    attn_iter(2)
    qkv_half(3, 1, alt_pre=True)
    rope(3, 1, nc.gpsimd)
    attn_iter(3)
    repack(1024, 512)
    attn_iter(4)
    attn_iter(5)
    repack(1536, 512)
    for qt in range(6, QT):
        attn_iter(qt)
    emit_tp(QT - 1)
    emit_outproj(QT - 1)


def _build_program():
    import concourse.tile as tile
    from concourse import bacc, mybir

    bf16 = mybir.dt.bfloat16

    nc = bacc.Bacc("TRN2", target_bir_lowering=False, debug=False,
                   num_devices=N_CORES)
    aps = {
        "xT": nc.dram_tensor("xT", [C, T], bf16, kind="ExternalInput").ap(),
        "wT": nc.dram_tensor("wT", [C, 768], bf16, kind="ExternalInput").ap(),
        "woT": nc.dram_tensor("woT", [FQ, C], bf16, kind="ExternalInput").ap(),
        "cos4": nc.dram_tensor("cos4", [128, T], bf16, kind="ExternalInput").ap(),
        "sin4": nc.dram_tensor("sin4", [128, T], bf16, kind="ExternalInput").ap(),
        "amask01": nc.dram_tensor("amask01", [128, 256], bf16, kind="ExternalInput").ap(),
        "ident": nc.dram_tensor("ident", [128, 128], bf16, kind="ExternalInput").ap(),
        "y": nc.dram_tensor("y", [T, C], bf16, kind="ExternalOutput").ap(),
    }
    from contextlib import ExitStack

    with tile.TileContext(nc) as tc, ExitStack() as ctx:
        _emit(nc, tc, aps, ctx)
    nc.compile()
    return nc


def _get_program():
    global _PROGRAM
    if _PROGRAM is None:
        _PROGRAM = _build_program()
    return _PROGRAM


def _host_inputs(x, w_qkv, w_out):
    import ml_dtypes

    bf16 = ml_dtypes.bfloat16
    x = np.asarray(x, np.float32)
    w_qkv = np.asarray(w_qkv, np.float32)
    w_out = np.asarray(w_out, np.float32)

    wq, wk, wv = w_qkv[0:C], w_qkv[C:2 * C], w_qkv[2 * C:3 * C]
    scale = 1.0 / math.sqrt(DH)

    # RoPE tables (transposed, tiled over the 4 heads of a block)
    inv_freq = 1.0 / (10000.0 ** (np.arange(0, DH, 2, dtype=np.float32) / DH))
    freqs = np.outer(np.arange(T, dtype=np.float32), inv_freq)  # [T, 32]
    cos4 = np.ascontiguousarray(np.tile(np.cos(freqs).T, (4, 1))).astype(bf16)
    sin4 = np.ascontiguousarray(np.tile(np.sin(freqs).T, (4, 1))).astype(bf16)

    # multiplicative 0/1 band masks for TRANSPOSED probabilities pT[k, q]:
    # [block kt=qt-2: allowed qq < kk | block kt=qt: allowed qq >= kk]
    i = np.arange(128)[:, None]  # kk (partitions)
    c = np.arange(128)[None, :]  # qq (free)
    m_first = (c < i).astype(np.float32)
    m_last = (c >= i).astype(np.float32)
    amask01 = np.ascontiguousarray(
        np.concatenate([m_first, m_last], axis=1)).astype(bf16)
    ident = np.eye(128, dtype=np.float32).astype(bf16)

    xT = [np.ascontiguousarray(x[b].T).astype(bf16) for b in range(B)]

    in_maps = []
    for core in range(N_CORES):
        b, g = divmod(core, 4)
        hs = range(4 * g, 4 * g + 4)
        rows = []
        for half in range(2):  # q_x1, q_x2
            rows.append(np.concatenate(
                [wq[h * DH + 32 * half:h * DH + 32 * half + 32] for h in hs]) * scale)
        for half in range(2):  # k_x1, k_x2
            rows.append(np.concatenate(
                [wk[h * DH + 32 * half:h * DH + 32 * half + 32] for h in hs]))
        rows.append(wv[g * FQ:(g + 1) * FQ])
        wmat = np.concatenate(rows)  # [768, C]
        wT = np.ascontiguousarray(wmat.T).astype(bf16)
        woT = np.ascontiguousarray(w_out[:, g * FQ:(g + 1) * FQ].T).astype(bf16)
        in_maps.append({
            "xT": xT[b], "wT": wT, "woT": woT,
            "cos4": cos4, "sin4": sin4, "amask01": amask01, "ident": ident,
        })
    return in_maps


def kernel(x, w_qkv, w_out, _trace=False):
    from concourse import bass_utils

    nc = _get_program()
    in_maps = _host_inputs(x, w_qkv, w_out)
    res = bass_utils.run_bass_kernel_spmd(
        nc, in_maps, core_ids=list(range(N_CORES)), trace=_trace,
    )
    parts = [np.asarray(res.results[core]["y"], dtype=np.float32)
             for core in range(N_CORES)]
    out = np.stack([
        parts[0] + parts[1] + parts[2] + parts[3],
        parts[4] + parts[5] + parts[6] + parts[7],
    ])
    if _trace:
        return out, res
    return out
